# revision 6
# baseline (speedup 1.0000x reference)
"""Trainium2 Bass kernel for BatchedFerroelectricBasis.

Math: the reference scans over the batch dim b with per-element state
bs[i,o,n].  Each step is elementwise-LINEAR in bs:

    su = u_b[i] * sigmoid(10*(x_b[i] - Ec))          u_b = sigmoid(10*(x_b - x_{b-1}))
    sl = (1-u_b[i]) * sigmoid(10*(-x_b[i] - Ec))
    bs <- (1 - 0.2*(su+sl)) * bs + 0.2*(su - sl)     (alpha = 0.8)
    out[b,o] = sum_{i,n} coef*Ps*tanh(k*(x_b[i] + Ec*bs)) + sum_{i,n} coef*bias

So per (i,o,n) element the recurrence is bs_b = A_b*bs_{b-1} + B_b, which maps
exactly onto the DVE `tensor_tensor_scan` instruction (state = (d0 * state) + d1
along the free axis).  Layout: partition = i (128), free = b (256); one chunk per
(o_local, n) pair; out_dim is sharded 8 ways across cores (16 o's per core).

Per chunk (all tiles [128, 256] f32):
  ACT: cpcn = sigmoid([10x | -10x] + bias(-10Ec))          (one [128,512] act)
  DVE: pq   = cpcn * [-0.2u | -0.2(1-u)]                   (tensor_tensor)
  DVE: A    = (p' + 1) + q'      = 1 - 0.2*(su+sl)         (scalar_tensor_tensor)
  DVE: Bt   = q' - p'            = 0.2*(su-sl)             (tensor_tensor sub)
  DVE: bs   = scan(A, Bt, init=1)                          (tensor_tensor_scan)
  DVE: targ = bs*Ec + x                                    (scalar_tensor_tensor)
  ACT: th   = tanh(k * targ)                               (act, per-partition scale)
  PE : psum[o,:] (+)= cP[:,c]^T @ th                       (matmul, accum over n)

The constant term sum(coef*bias) and the final transpose/concat are folded in on
the host (they are b-independent / pure data movement).
"""

import os
import sys
from contextlib import ExitStack

import numpy as np

for _p in ("/root/.axon_site", "/root/.axon_site/_ro/trn_rl_repo", "/opt/trn_rl_repo"):
    if os.path.isdir(_p) and _p not in sys.path:
        sys.path.append(_p)

import concourse.bass as bass
import concourse.tile as tile
from concourse import bacc, mybir
from concourse.bass_utils import run_bass_kernel_spmd

B, I, O, N = 256, 128, 128, 16
NCORES = 8
OL = O // NCORES  # 16 out-dims per core
F32 = mybir.dt.float32

LAST_RESULTS = None  # BassKernelResults of the most recent run (for test.py)

_prog_cache = {}


def _build_program():
    nc = bacc.Bacc("TRN2", target_bir_lowering=False, debug=False)

    xTpm_d = nc.dram_tensor("xTpm", [I, 2 * B], F32, kind="ExternalInput").ap()
    U2_d = nc.dram_tensor("U2", [I, 2 * B], F32, kind="ExternalInput").ap()
    xT_d = nc.dram_tensor("xT", [I, B], F32, kind="ExternalInput").ap()
    Ec_d = nc.dram_tensor("EcS", [I, OL * N], F32, kind="ExternalInput").ap()
    bEc_d = nc.dram_tensor("bEcS", [I, OL * N], F32, kind="ExternalInput").ap()
    k_d = nc.dram_tensor("kS", [I, OL * N], F32, kind="ExternalInput").ap()
    cP_d = nc.dram_tensor("cPS", [I, OL * N], F32, kind="ExternalInput").ap()
    out_d = nc.dram_tensor("outT", [1, OL * B], F32, kind="ExternalOutput").ap()

    with tile.TileContext(nc) as tc, ExitStack() as ctx:
        pers = ctx.enter_context(tc.tile_pool(name="pers", bufs=1))
        work = ctx.enter_context(tc.tile_pool(name="work", bufs=4))
        psum = ctx.enter_context(tc.tile_pool(name="psum", bufs=1, space="PSUM"))

        xTpm = pers.tile([I, 2 * B], F32, name="xTpm_s")
        nc.gpsimd.dma_start(xTpm[:], xTpm_d[:])
        U2 = pers.tile([I, 2 * B], F32, name="U2_s")
        nc.gpsimd.dma_start(U2[:], U2_d[:])
        xT = pers.tile([I, B], F32, name="xT_s")
        nc.gpsimd.dma_start(xT[:], xT_d[:])
        EcS = pers.tile([I, OL * N], F32, name="EcS_s")
        nc.gpsimd.dma_start(EcS[:], Ec_d[:])
        bEcS = pers.tile([I, OL * N], F32, name="bEcS_s")
        nc.gpsimd.dma_start(bEcS[:], bEc_d[:])
        kS = pers.tile([I, OL * N], F32, name="kS_s")
        nc.gpsimd.dma_start(kS[:], k_d[:])
        cPS = pers.tile([I, OL * N], F32, name="cPS_s")
        nc.gpsimd.dma_start(cPS[:], cP_d[:])

        acc = psum.tile([1, OL * B], F32, name="acc")
        outs = pers.tile([1, OL * B], F32, name="outs")

        for o in range(OL):
            cols = [o * N + n for n in range(N)]

            # --- ACT phase 1: both sigmoids for all chunks of this group ---
            cpcn_t = []
            for n in range(N):
                c = cols[n]
                t = work.tile([I, 2 * B], F32, name=f"cpcn_{o}_{n}", tag="cpcn", bufs=18)
                nc.scalar.activation(
                    t[:], xTpm[:], mybir.ActivationFunctionType.Sigmoid,
                    bias=bEcS[:, c : c + 1], scale=1.0,
                )
                cpcn_t.append(t)

            # --- DVE phase: coefficients + scan + tanh argument ---
            targ_t = []
            for n in range(N):
                c = cols[n]
                cpcn = cpcn_t[n]
                pq = work.tile([I, 2 * B], F32, name=f"pq_{o}_{n}", tag="pq", bufs=4)
                nc.vector.tensor_tensor(pq[:], cpcn[:], U2[:], mybir.AluOpType.mult)
                At = work.tile([I, B], F32, name=f"A_{o}_{n}", tag="A", bufs=4)
                nc.vector.scalar_tensor_tensor(
                    At[:], pq[:, 0:B], 1.0, pq[:, B : 2 * B],
                    mybir.AluOpType.add, mybir.AluOpType.add,
                )
                Bt = work.tile([I, B], F32, name=f"B_{o}_{n}", tag="Bt", bufs=4)
                nc.vector.tensor_tensor(
                    Bt[:], pq[:, B : 2 * B], pq[:, 0:B], mybir.AluOpType.subtract
                )
                bs = work.tile([I, B], F32, name=f"bs_{o}_{n}", tag="bs", bufs=4)
                nc.vector.tensor_tensor_scan(
                    bs[:], At[:], Bt[:], 1.0,
                    mybir.AluOpType.mult, mybir.AluOpType.add,
                )
                targ = work.tile([I, B], F32, name=f"targ_{o}_{n}", tag="targ", bufs=18)
                nc.vector.scalar_tensor_tensor(
                    targ[:], bs[:], EcS[:, c : c + 1], xT[:],
                    mybir.AluOpType.mult, mybir.AluOpType.add,
                )
                targ_t.append(targ)

            # --- ACT phase 2: tanh for all chunks of this group ---
            th_t = []
            for n in range(N):
                c = cols[n]
                th = work.tile([I, B], F32, name=f"th_{o}_{n}", tag="th", bufs=18)
                nc.scalar.activation(
                    th[:], targ_t[n][:], mybir.ActivationFunctionType.Tanh,
                    bias=0.0, scale=kS[:, c : c + 1],
                )
                th_t.append(th)

            # --- PE phase: weighted i-contraction, accumulate over n ---
            for n in range(N):
                c = cols[n]
                nc.tensor.matmul(
                    acc[0:1, o * B : (o + 1) * B], cPS[:, c : c + 1], th_t[n][:],
                    start=(n == 0), stop=(n == N - 1),
                )

        nc.scalar.copy(outs[:], acc[:])
        nc.gpsimd.dma_start(out_d[:], outs[:])

    nc.compile()
    return nc


def _sigmoid(z):
    return 1.0 / (1.0 + np.exp(-z))


def make_in_maps(x, k, Ec, Ps, bias, coef):
    x = np.ascontiguousarray(x, dtype=np.float32)
    xT = np.ascontiguousarray(x.T)  # [I, B]
    xTpm = np.concatenate([10.0 * xT, -10.0 * xT], axis=1)  # [I, 2B]

    dx = x - np.vstack([np.zeros((1, I), np.float32), x[:-1]])
    u = _sigmoid(10.0 * dx).astype(np.float32)  # [B, I]
    U2 = np.concatenate([(-0.2 * u).T, (-0.2 * (1.0 - u)).T], axis=1)  # [I, 2B]

    cP = (coef * Ps).astype(np.float32)  # [I, O, N]
    in_maps = []
    for core in range(NCORES):
        sl = slice(core * OL, (core + 1) * OL)
        EcS = np.ascontiguousarray(Ec[:, sl, :].reshape(I, OL * N), dtype=np.float32)
        in_maps.append({
            "xTpm": np.ascontiguousarray(xTpm, dtype=np.float32),
            "U2": np.ascontiguousarray(U2, dtype=np.float32),
            "xT": xT.astype(np.float32),
            "EcS": EcS,
            "bEcS": np.ascontiguousarray(-10.0 * EcS),
            "kS": np.ascontiguousarray(k[:, sl, :].reshape(I, OL * N), dtype=np.float32),
            "cPS": np.ascontiguousarray(cP[:, sl, :].reshape(I, OL * N)),
        })
    return in_maps


def _ensure_ntff_hook():
    """The agent image's antenv lacks axon_hooks; shim it so trace=True works."""
    try:
        import antenv.axon_hooks  # noqa: F401
        return
    except ImportError:
        pass
    import types

    import antenv
    try:
        from trn_agent_boot.trn_boot import _ntff_profile_via_ctypes
    except ImportError:
        return
    mod = types.ModuleType("antenv.axon_hooks")
    state = {"h": None}
    mod.set_axon_ntff_profile_hook = lambda h: state.__setitem__("h", h)
    mod.get_axon_ntff_profile_hook = lambda: state["h"]
    sys.modules["antenv.axon_hooks"] = mod
    antenv.axon_hooks = mod
    so = "/opt/axon/libaxon_pjrt.so"
    if os.path.exists(so):
        mod.set_axon_ntff_profile_hook(_ntff_profile_via_ctypes(so))


def kernel(x, k, Ec, Ps, bias, coef, trace=False):
    global LAST_RESULTS
    if trace:
        _ensure_ntff_hook()
    if "prog" not in _prog_cache:
        _prog_cache["prog"] = _build_program()
    nc = _prog_cache["prog"]

    in_maps = make_in_maps(x, k, Ec, Ps, bias, coef)
    res = run_bass_kernel_spmd(nc, in_maps, list(range(NCORES)), trace=trace)
    LAST_RESULTS = res

    cb = (np.asarray(coef, np.float64) * np.asarray(bias, np.float64)).sum(axis=(0, 2))
    out = np.empty((B, O), dtype=np.float32)
    for core in range(NCORES):
        sl = slice(core * OL, (core + 1) * OL)
        out[:, sl] = res.results[core]["outT"].reshape(OL, B).T + cb[None, sl].astype(
            np.float32
        )
    return out
